# revision 2
# baseline (speedup 1.0000x reference)
"""Butterfly permuter kernel for Trainium2 (8 NeuronCores, SPMD data-parallel).

The reference applies 10 butterfly rotation stages along the feature axis
(dim=1024) of x [16384, 1024].  Each row is transformed independently, and the
10 stages compose into a single dense 1024x1024 orthogonal matrix R with
y_rows = x_rows @ R.  We compute R on the host in float64 from `angles`, then
run a tiled bf16 matmul on each core (fp32 PSUM accumulation; rel-err budget
is 2e-2 and bf16 lands ~2e-3 while doubling PE stream rate vs fp32):

  per core: x_shard [2048, 1024]
  - SWDGE-DMA x in 1 MiB chunks with inline fp32->bf16 cast
    [128 part, subtiles x 1024] (cast during DMA frees the compute engines)
  - PE-transpose each [128 tok, 128 dim] bf16 block (via bf16 identity) to
    get X^T blocks (contraction dim on partitions), bf16 PSUM, evacuate
    PSUM->SBUF on ScalarE
  - 16 accumulating bf16 matmuls per subtile: psum_y[jh] += XT_kb^T @ R_kb
    (bf16 moving operand streams 2 elem/cycle - 2x fp32 PE rate)
  - evacuate y PSUM->SBUF (fp32) on VectorE, HWDGE-DMA out 1 MiB chunks

Inputs arrive full-size; sharding is across the token axis (2048 rows/core).
"""

import numpy as np

import concourse.bass as bass
import concourse.mybir as mybir
import concourse.tile as tile
from concourse import bacc
from concourse.bass_utils import run_bass_kernel_spmd

N_CORES = 8
DIM = 1024
NUM_STAGES = 10
N_TOKENS = 16384
TOK_PER_CORE = N_TOKENS // N_CORES  # 2048
SUB = 128  # tokens per subtile (partition dim)
KB = DIM // 128  # 8 contraction blocks

F32 = mybir.dt.float32
BF16 = mybir.dt.bfloat16
NP_BF16 = mybir.dt.np(BF16)


def compose_transform(angles: np.ndarray) -> np.ndarray:
    """Compose the 10 butterfly stages into R (float32) with y = x @ R."""
    y = np.eye(DIM, dtype=np.float64)
    a = np.asarray(angles, dtype=np.float64)
    for s in range(NUM_STAGES):
        span = 2 ** (s + 1)
        half = span // 2
        y = y.reshape(-1, DIM // span, span)
        left, right = y[..., :half], y[..., half:]
        th = a[s].reshape(1, DIM // span, half)
        c, sn = np.cos(th), np.sin(th)
        y = np.concatenate([c * left + sn * right, -sn * left + c * right], -1)
        y = y.reshape(-1, DIM)
    # row t of y is transform(e_t), so transform(x) = x @ y
    return np.ascontiguousarray(y, dtype=np.float32)


def build_bass(reps: int = 1):
    """reps>1 repeats the whole pipeline in one NEFF (for marginal timing)."""
    nc = bacc.Bacc(None, target_bir_lowering=False)
    x = nc.dram_tensor("x", [TOK_PER_CORE, DIM], F32, kind="ExternalInput")
    w = nc.dram_tensor("w", [DIM, DIM], BF16, kind="ExternalInput")
    ident = nc.dram_tensor("ident", [128, 128], BF16, kind="ExternalInput")
    y = nc.dram_tensor("y", [TOK_PER_CORE, DIM], F32, kind="ExternalOutput")

    n_sub = TOK_PER_CORE // SUB  # 16 subtiles of 128 tokens

    # Variable-size DMA chunking (in units of 128-token subtiles): small
    # chunks at the start for a fast pipeline ramp, small at the end for a
    # short drain; 2-subtile chunks in steady state.
    in_chunks = [1, 1, 2, 2, 2, 2, 2, 2, 2]
    out_chunks = [2, 2, 2, 2, 2, 2, 2, 1, 1]
    assert sum(in_chunks) == n_sub and sum(out_chunks) == n_sub
    in_start = [sum(in_chunks[:i]) for i in range(len(in_chunks))]
    out_start = [sum(out_chunks[:i]) for i in range(len(out_chunks))]
    sub_to_in_chunk = {}
    for ci, (st, ln) in enumerate(zip(in_start, in_chunks)):
        for s in range(st, st + ln):
            sub_to_in_chunk[s] = ci
    sub_to_out_chunk = {}
    for ci, (st, ln) in enumerate(zip(out_start, out_chunks)):
        for s in range(st, st + ln):
            sub_to_out_chunk[s] = ci

    with tile.TileContext(nc) as tc:
        with (
            tc.tile_pool(name="const", bufs=1) as const_pool,
            tc.tile_pool(name="xin", bufs=3) as xin_pool,
            tc.tile_pool(name="xt", bufs=5) as xt_pool,
            tc.tile_pool(name="yout", bufs=3) as yout_pool,
            tc.tile_pool(name="pst", bufs=4, space="PSUM") as pst_pool,
            tc.tile_pool(name="psy", bufs=4, space="PSUM") as psy_pool,
        ):
            ident_sb = const_pool.tile([128, 128], BF16, name="ident_sb")
            nc.sync.dma_start(ident_sb[:], ident[:])

            x_tiles = [None] * len(in_chunks)  # chunk idx -> tile
            y_tiles = [None] * len(out_chunks)

            def load_chunk(ci):
                st, ln = in_start[ci], in_chunks[ci]
                x_tile = xin_pool.tile([128, ln * DIM], BF16, name="x_chunk",
                                       tag="x_chunk",
                                       padded_shape=[128, 2 * DIM])
                r0 = st * SUB
                # SWDGE: fp32 DRAM -> bf16 SBUF cast happens in the SDMA
                # datapath; HBM side still reads the full fp32 bytes.
                nc.gpsimd.dma_start(
                    x_tile[:, : ln * DIM].rearrange("p (s c) -> p s c", c=DIM),
                    x[r0 : r0 + ln * SUB, :].rearrange("(s p) c -> p s c", p=128),
                )
                x_tiles[ci] = x_tile

            load_chunk(0)
            first_load_done = True

            # W arrives bf16 from the host: one HWDGE DMA per j-half (1 MiB
            # each, j-half-major so the first MiB unblocks the jh0 matmuls).
            w_sb = const_pool.tile([128, KB * DIM], BF16, name="w_sb")

            def w_off(jh, kb):
                return (jh * KB + kb) * 512

            for jh in range(2):
                nc.sync.dma_start(
                    w_sb[:, jh * KB * 512 : (jh + 1) * KB * 512].rearrange(
                        "p (kb c) -> p kb c", c=512
                    ),
                    w[:, jh * 512 : (jh + 1) * 512].rearrange(
                        "(kb p) c -> p kb c", p=128
                    ),
                )

            xts = [None] * n_sub

            def emit_transpose(s):
                ci = sub_to_in_chunk[s]
                xcol = (s - in_start[ci]) * DIM
                x_tile = x_tiles[ci]
                ps_t0 = pst_pool.tile([128, 512], BF16, name="ps_t0", tag="ps_t")
                ps_t1 = pst_pool.tile([128, 512], BF16, name="ps_t1", tag="ps_t")
                for kb in range(KB):
                    dst = ps_t0 if kb < 4 else ps_t1
                    j = (kb % 4) * 128
                    nc.tensor.transpose(
                        dst[:, j : j + 128],
                        x_tile[:, xcol + kb * 128 : xcol + (kb + 1) * 128],
                        ident_sb,
                    )
                xt = xt_pool.tile([128, DIM], BF16, name="xt", tag="xt")
                nc.scalar.copy(xt[:, :512], ps_t0[:])
                nc.scalar.copy(xt[:, 512:], ps_t1[:])
                xts[s] = xt

            def emit_matmul(s, jh):
                co = sub_to_out_chunk[s]
                st, ln = out_start[co], out_chunks[co]
                if s == st and jh == 0:
                    y_tiles[co] = yout_pool.tile(
                        [128, ln * DIM], F32, name="y_chunk", tag="y_chunk",
                        padded_shape=[128, 2 * DIM],
                    )
                y_tile = y_tiles[co]
                ycol = (s - st) * DIM + jh * 512
                xt = xts[s]
                ps_y = psy_pool.tile([128, 512], F32, name="ps_y", tag="ps_y")
                for kb in range(KB):
                    off = w_off(jh, kb)
                    nc.tensor.matmul(
                        ps_y[:],
                        xt[:, kb * 128 : (kb + 1) * 128],
                        w_sb[:, off : off + 512],
                        start=(kb == 0),
                        stop=(kb == KB - 1),
                    )
                nc.vector.tensor_copy(y_tile[:, ycol : ycol + 512], ps_y[:])
                if s == st + ln - 1 and jh == 1:
                    r0 = st * SUB
                    # y stores go out on the ACT HWDGE ring so they don't
                    # queue ahead of the W loads on the SP ring.
                    nc.scalar.dma_start(
                        y[r0 : r0 + ln * SUB, :].rearrange("(s p) c -> p s c", p=128),
                        y_tile[:, : ln * DIM].rearrange("p (s c) -> p s c", c=DIM),
                    )

            # Skewed software pipeline: transposes run two subtiles ahead of
            # the matmuls (PE runway while W streams in), and j-halves are
            # staggered one subtile apart: MM(s, jh0) then MM(s-1, jh1), so
            # subtile 0's jh1 group (which needs the second half of W)
            # doesn't stall the in-order PE stream at startup.
            SKEW = 2
            for _rep in range(reps):
                if not first_load_done:
                    load_chunk(0)
                first_load_done = False
                for p in range(min(SKEW, n_sub)):
                    ci = sub_to_in_chunk[p]
                    if p == in_start[ci] and p > 0:
                        load_chunk(ci)
                    emit_transpose(p)
                for s in range(n_sub):
                    nxt = s + SKEW
                    if nxt < n_sub:
                        ci = sub_to_in_chunk[nxt]
                        if nxt == in_start[ci]:
                            load_chunk(ci)
                        emit_transpose(nxt)
                    emit_matmul(s, 0)
                    if s >= 1:
                        emit_matmul(s - 1, 1)
                emit_matmul(n_sub - 1, 1)
    nc.compile()
    return nc


_NC_CACHE = None


def _get_nc():
    global _NC_CACHE
    if _NC_CACHE is None:
        _NC_CACHE = build_bass()
    return _NC_CACHE


def host_inputs(x: np.ndarray, angles: np.ndarray):
    """Per-core input maps (x fp32 shards; w/ident pre-cast to bf16)."""
    x = np.ascontiguousarray(np.asarray(x, dtype=np.float32))
    w = compose_transform(angles).astype(NP_BF16)
    ident = np.eye(128, dtype=np.float32).astype(NP_BF16)
    return [
        {
            "x": x[c * TOK_PER_CORE : (c + 1) * TOK_PER_CORE],
            "w": w,
            "ident": ident,
        }
        for c in range(N_CORES)
    ]


def run(x: np.ndarray, angles: np.ndarray, trace: bool = False):
    """Run on 8 cores; returns (y_full, BassKernelResults)."""
    nc = _get_nc()
    in_maps = host_inputs(x, angles)
    res = run_bass_kernel_spmd(
        nc, in_maps, core_ids=list(range(N_CORES)), trace=trace
    )
    y = np.concatenate([res.results[c]["y"] for c in range(N_CORES)], axis=0)
    return y, res


def kernel(x: np.ndarray, angles: np.ndarray) -> np.ndarray:
    y, _ = run(x, angles, trace=False)
    return y


# revision 3
# speedup vs baseline: 1.2800x; 1.2800x over previous
"""Butterfly permuter kernel for Trainium2 (8 NeuronCores, SPMD data-parallel).

The reference applies 10 butterfly rotation stages along the feature axis
(dim=1024) of x [16384, 1024].  Stages 1-9 (spans 2..512) only mix within the
two 512-wide halves of the feature axis, so their composition is
blockdiag(A, B) with dense 512x512 blocks; stage 10 (span 1024) rotates
element j with element j+512 using per-column angles.  We therefore compute

    z0 = x[:, :512] @ A          (PE, bf16, fp32 PSUM accumulation)
    z1 = x[:, 512:] @ B
    y[:, :512] = c*z0 + s*z1     (DVE/GpSimd elementwise, c/s = cos/sin of
    y[:, 512:] = c*z1 - s*z0      angles[9], broadcast along tokens)

which halves the PE matmul work vs the dense 1024x1024 formulation (the PE
moving-operand stream at 1 elem/cycle was the bottleneck) and puts the last
stage on the otherwise-idle vector engines.  bf16 operands (rel-err budget
2e-2; this lands ~2e-3) with fp32 accumulation and fp32 stage-10 arithmetic.

Per core: x_shard [2048, 1024]
  - SWDGE-DMA x chunks with inline fp32->bf16 cast
  - PE-transpose each [128 tok, 128 dim] bf16 block (bf16 identity, bf16
    PSUM) to get X^T blocks, evacuate PSUM->SBUF on ScalarE
  - per 128-token subtile: 2 accumulation groups of 4 bf16 matmuls
    (z0 += XT_kb^T @ A_kb, z1 += XT_kb^T @ B_kb), N=512
  - stage 10: 4 DVE mults (PSUM x coeff -> SBUF temps) + 2 GpSimd add/sub
    (temps -> y tile, fp32)
  - HWDGE-DMA y out in chunks on the ACT ring

Inputs arrive full-size; sharding is across the token axis (2048 rows/core).
"""

import numpy as np

import concourse.bass as bass
import concourse.mybir as mybir
import concourse.tile as tile
from concourse import bacc
from concourse.bass_utils import run_bass_kernel_spmd

N_CORES = 8
DIM = 1024
NUM_STAGES = 10
N_TOKENS = 16384
TOK_PER_CORE = N_TOKENS // N_CORES  # 2048
SUB = 128  # tokens per subtile (partition dim)
KB = DIM // 128  # 8 contraction blocks
HALF = DIM // 2

F32 = mybir.dt.float32
BF16 = mybir.dt.bfloat16
NP_BF16 = mybir.dt.np(BF16)
MULT = mybir.AluOpType.mult
ADD = mybir.AluOpType.add
SUBTRACT = mybir.AluOpType.subtract


def _compose(angles: np.ndarray, stages) -> np.ndarray:
    y = np.eye(DIM, dtype=np.float64)
    a = np.asarray(angles, dtype=np.float64)
    for s in stages:
        span = 2 ** (s + 1)
        half = span // 2
        y = y.reshape(-1, DIM // span, span)
        left, right = y[..., :half], y[..., half:]
        th = a[s].reshape(1, DIM // span, half)
        c, sn = np.cos(th), np.sin(th)
        y = np.concatenate([c * left + sn * right, -sn * left + c * right], -1)
        y = y.reshape(-1, DIM)
    return y


def compose_transform(angles: np.ndarray) -> np.ndarray:
    """Full dense R (float32) with y = x @ R (kept for reference/tests)."""
    return np.ascontiguousarray(_compose(angles, range(NUM_STAGES)),
                                dtype=np.float32)


def build_bass(reps: int = 1):
    """reps>1 repeats the whole pipeline in one NEFF (for marginal timing)."""
    nc = bacc.Bacc(None, target_bir_lowering=False)
    x = nc.dram_tensor("x", [TOK_PER_CORE, DIM], F32, kind="ExternalInput")
    # rows 0-511: A = R_{1..9}[:512, :512]; rows 512-1023: B (bottom-right)
    wlow = nc.dram_tensor("wlow", [DIM, HALF], BF16, kind="ExternalInput")
    # stage-10 cos/sin, replicated to 128 partitions on the host
    cvec = nc.dram_tensor("cvec", [128, HALF], F32, kind="ExternalInput")
    svec = nc.dram_tensor("svec", [128, HALF], F32, kind="ExternalInput")
    ident = nc.dram_tensor("ident", [128, 128], BF16, kind="ExternalInput")
    y = nc.dram_tensor("y", [TOK_PER_CORE, DIM], F32, kind="ExternalOutput")

    n_sub = TOK_PER_CORE // SUB  # 16 subtiles of 128 tokens

    in_chunks = [1, 1, 2, 2, 2, 2, 2, 2, 2]
    out_chunks = [2, 2, 2, 2, 2, 2, 2, 1, 1]
    assert sum(in_chunks) == n_sub and sum(out_chunks) == n_sub
    in_start = [sum(in_chunks[:i]) for i in range(len(in_chunks))]
    out_start = [sum(out_chunks[:i]) for i in range(len(out_chunks))]
    sub_to_in_chunk = {}
    for ci, (st, ln) in enumerate(zip(in_start, in_chunks)):
        for s in range(st, st + ln):
            sub_to_in_chunk[s] = ci
    sub_to_out_chunk = {}
    for ci, (st, ln) in enumerate(zip(out_start, out_chunks)):
        for s in range(st, st + ln):
            sub_to_out_chunk[s] = ci

    with tile.TileContext(nc) as tc:
        with (
            tc.tile_pool(name="const", bufs=1) as const_pool,
            tc.tile_pool(name="xin", bufs=3) as xin_pool,
            tc.tile_pool(name="xt", bufs=5) as xt_pool,
            tc.tile_pool(name="tmp", bufs=8) as tmp_pool,
            tc.tile_pool(name="yout", bufs=3) as yout_pool,
            tc.tile_pool(name="pst", bufs=4, space="PSUM") as pst_pool,
            tc.tile_pool(name="psz", bufs=4, space="PSUM") as psz_pool,
        ):
            ident_sb = const_pool.tile([128, 128], BF16, name="ident_sb")
            nc.sync.dma_start(ident_sb[:], ident[:])

            x_tiles = [None] * len(in_chunks)
            y_tiles = [None] * len(out_chunks)

            def load_chunk(ci):
                st, ln = in_start[ci], in_chunks[ci]
                x_tile = xin_pool.tile([128, ln * DIM], BF16, name="x_chunk",
                                       tag="x_chunk",
                                       padded_shape=[128, 2 * DIM])
                r0 = st * SUB
                # SWDGE: fp32 DRAM -> bf16 SBUF cast in the SDMA datapath.
                nc.gpsimd.dma_start(
                    x_tile[:, : ln * DIM].rearrange("p (s c) -> p s c", c=DIM),
                    x[r0 : r0 + ln * SUB, :].rearrange("(s p) c -> p s c", p=128),
                )
                x_tiles[ci] = x_tile

            load_chunk(0)
            first_load_done = True

            # A/B blocks: [128, kb*512] with a_sb[p, kb*512+j] = A[kb*128+p, j]
            a_sb = const_pool.tile([128, 4 * HALF], BF16, name="a_sb")
            b_sb = const_pool.tile([128, 4 * HALF], BF16, name="b_sb")
            for dst, r0 in ((a_sb, 0), (b_sb, HALF)):
                nc.sync.dma_start(
                    dst[:].rearrange("p (kb c) -> p kb c", c=HALF),
                    wlow[r0 : r0 + HALF, :].rearrange("(kb p) c -> p kb c", p=128),
                )
            c_sb = const_pool.tile([128, HALF], F32, name="c_sb")
            s_sb = const_pool.tile([128, HALF], F32, name="s_sb")
            nc.sync.dma_start(c_sb[:], cvec[:])
            nc.sync.dma_start(s_sb[:], svec[:])

            xts = [None] * n_sub

            def emit_transpose(s):
                ci = sub_to_in_chunk[s]
                xcol = (s - in_start[ci]) * DIM
                x_tile = x_tiles[ci]
                ps_t0 = pst_pool.tile([128, 512], BF16, name="ps_t0", tag="ps_t")
                ps_t1 = pst_pool.tile([128, 512], BF16, name="ps_t1", tag="ps_t")
                for kb in range(KB):
                    dst = ps_t0 if kb < 4 else ps_t1
                    j = (kb % 4) * 128
                    nc.tensor.transpose(
                        dst[:, j : j + 128],
                        x_tile[:, xcol + kb * 128 : xcol + (kb + 1) * 128],
                        ident_sb,
                    )
                xt = xt_pool.tile([128, DIM], BF16, name="xt", tag="xt")
                nc.scalar.copy(xt[:, :512], ps_t0[:])
                nc.scalar.copy(xt[:, 512:], ps_t1[:])
                xts[s] = xt

            def emit_subtile(s):
                co = sub_to_out_chunk[s]
                st, ln = out_start[co], out_chunks[co]
                if s == st:
                    y_tiles[co] = yout_pool.tile(
                        [128, ln * DIM], F32, name="y_chunk", tag="y_chunk",
                        padded_shape=[128, 2 * DIM],
                    )
                y_tile = y_tiles[co]
                ycol = (s - st) * DIM
                xt = xts[s]
                # z0 = sum_kb XT_kb^T @ A_kb ; z1 = sum_kb XT_{kb+4}^T @ B_kb
                z0 = psz_pool.tile([128, 512], F32, name="z0", tag="ps_z")
                z1 = psz_pool.tile([128, 512], F32, name="z1", tag="ps_z")
                for zi, (z, w_sb) in enumerate(((z0, a_sb), (z1, b_sb))):
                    for k in range(4):
                        kb = zi * 4 + k
                        nc.tensor.matmul(
                            z[:],
                            xt[:, kb * 128 : (kb + 1) * 128],
                            w_sb[:, k * 512 : (k + 1) * 512],
                            start=(k == 0),
                            stop=(k == 3),
                        )
                # stage 10: y0 = c*z0 + s*z1 ; y1 = c*z1 - s*z0
                t_c0 = tmp_pool.tile([128, 512], F32, name="t_c0", tag="tmp")
                t_s0 = tmp_pool.tile([128, 512], F32, name="t_s0", tag="tmp")
                t_c1 = tmp_pool.tile([128, 512], F32, name="t_c1", tag="tmp")
                t_s1 = tmp_pool.tile([128, 512], F32, name="t_s1", tag="tmp")
                nc.vector.tensor_tensor(t_c0[:], z0[:], c_sb[:], MULT)
                nc.vector.tensor_tensor(t_s0[:], z0[:], s_sb[:], MULT)
                nc.vector.tensor_tensor(t_c1[:], z1[:], c_sb[:], MULT)
                nc.vector.tensor_tensor(t_s1[:], z1[:], s_sb[:], MULT)
                nc.gpsimd.tensor_tensor(
                    y_tile[:, ycol : ycol + 512], t_c0[:], t_s1[:], ADD)
                nc.gpsimd.tensor_tensor(
                    y_tile[:, ycol + 512 : ycol + 1024], t_c1[:], t_s0[:],
                    SUBTRACT)
                if s == st + ln - 1:
                    r0 = st * SUB
                    # y stores on the ACT HWDGE ring (x loads are SWDGE).
                    nc.scalar.dma_start(
                        y[r0 : r0 + ln * SUB, :].rearrange("(s p) c -> p s c", p=128),
                        y_tile[:, : ln * DIM].rearrange("p (s c) -> p s c", c=DIM),
                    )

            # Transposes run SKEW subtiles ahead of the matmuls so the PE
            # never waits on the ScalarE PSUM->SBUF evacuation of its own
            # transpose outputs.
            SKEW = 2
            for _rep in range(reps):
                if not first_load_done:
                    load_chunk(0)
                first_load_done = False
                for p in range(min(SKEW, n_sub)):
                    ci = sub_to_in_chunk[p]
                    if p == in_start[ci] and p > 0:
                        load_chunk(ci)
                    emit_transpose(p)
                for s in range(n_sub):
                    nxt = s + SKEW
                    if nxt < n_sub:
                        ci = sub_to_in_chunk[nxt]
                        if nxt == in_start[ci]:
                            load_chunk(ci)
                        emit_transpose(nxt)
                    emit_subtile(s)
    nc.compile()
    return nc


_NC_CACHE = None


def _get_nc():
    global _NC_CACHE
    if _NC_CACHE is None:
        _NC_CACHE = build_bass()
    return _NC_CACHE


def host_inputs(x: np.ndarray, angles: np.ndarray):
    """Per-core input maps (x fp32 shards; weights/coeffs precomputed)."""
    x = np.ascontiguousarray(np.asarray(x, dtype=np.float32))
    angles = np.asarray(angles)
    r9 = _compose(angles, range(NUM_STAGES - 1))
    wlow = np.concatenate([r9[:HALF, :HALF], r9[HALF:, HALF:]], axis=0)
    wlow = np.ascontiguousarray(wlow.astype(NP_BF16))
    c = np.cos(np.asarray(angles[NUM_STAGES - 1], dtype=np.float64))
    s = np.sin(np.asarray(angles[NUM_STAGES - 1], dtype=np.float64))
    cvec = np.ascontiguousarray(
        np.broadcast_to(c.astype(np.float32), (128, HALF)))
    svec = np.ascontiguousarray(
        np.broadcast_to(s.astype(np.float32), (128, HALF)))
    ident = np.eye(128, dtype=np.float32).astype(NP_BF16)
    return [
        {
            "x": x[c_ * TOK_PER_CORE : (c_ + 1) * TOK_PER_CORE],
            "wlow": wlow,
            "cvec": cvec,
            "svec": svec,
            "ident": ident,
        }
        for c_ in range(N_CORES)
    ]


def run(x: np.ndarray, angles: np.ndarray, trace: bool = False):
    """Run on 8 cores; returns (y_full, BassKernelResults)."""
    nc = _get_nc()
    in_maps = host_inputs(x, angles)
    res = run_bass_kernel_spmd(
        nc, in_maps, core_ids=list(range(N_CORES)), trace=trace
    )
    y = np.concatenate([res.results[c]["y"] for c in range(N_CORES)], axis=0)
    return y, res


def kernel(x: np.ndarray, angles: np.ndarray) -> np.ndarray:
    y, _ = run(x, angles, trace=False)
    return y


# revision 8
# speedup vs baseline: 1.4360x; 1.1218x over previous
"""Butterfly permuter kernel for Trainium2 (8 NeuronCores, SPMD data-parallel).

The reference applies 10 butterfly rotation stages along the feature axis
(dim=1024) of x [16384, 1024].  Stages 1-9 (spans 2..512) only mix within the
two 512-wide halves of the feature axis, so their composition is
blockdiag(A, B) with dense 512x512 blocks; stage 10 (span 1024) rotates
element j with element j+512 using per-column angles.  We therefore compute

    z0 = x[:, :512] @ A          (PE, bf16, fp32 PSUM accumulation)
    z1 = x[:, 512:] @ B
    y[:, :512] = c*z0 + s*z1     (DVE/GpSimd elementwise, c/s = cos/sin of
    y[:, 512:] = c*z1 - s*z0      angles[9], broadcast along tokens)

which halves the PE matmul work vs the dense 1024x1024 formulation (the PE
moving-operand stream at 1 elem/cycle was the bottleneck) and puts the last
stage on the otherwise-idle vector engines.  bf16 operands (rel-err budget
2e-2; this lands ~2e-3) with fp32 accumulation and fp32 stage-10 arithmetic.

Per core: x_shard [2048, 1024]
  - SWDGE-DMA x chunks with inline fp32->bf16 cast
  - PE-transpose each [128 tok, 128 dim] bf16 block (bf16 identity, bf16
    PSUM) to get X^T blocks, evacuate PSUM->SBUF on ScalarE
  - per 128-token subtile: 2 accumulation groups of 4 bf16 matmuls
    (z0 += XT_kb^T @ A_kb, z1 += XT_kb^T @ B_kb), N=512
  - stage 10: 4 DVE mults (PSUM x coeff -> SBUF temps) + 2 GpSimd add/sub
    (temps -> y tile, fp32)
  - HWDGE-DMA y out in chunks on the ACT ring

Inputs arrive full-size; sharding is across the token axis (2048 rows/core).
"""

import numpy as np

import concourse.bass as bass
import concourse.mybir as mybir
import concourse.tile as tile
from concourse import bacc
from concourse.bass_utils import run_bass_kernel_spmd

N_CORES = 8
DIM = 1024
NUM_STAGES = 10
N_TOKENS = 16384
TOK_PER_CORE = N_TOKENS // N_CORES  # 2048
SUB = 128  # tokens per subtile (partition dim)
KB = DIM // 128  # 8 contraction blocks
HALF = DIM // 2

F32 = mybir.dt.float32
BF16 = mybir.dt.bfloat16
NP_BF16 = mybir.dt.np(BF16)
MULT = mybir.AluOpType.mult
ADD = mybir.AluOpType.add
SUBTRACT = mybir.AluOpType.subtract


def _compose(angles: np.ndarray, stages) -> np.ndarray:
    y = np.eye(DIM, dtype=np.float64)
    a = np.asarray(angles, dtype=np.float64)
    for s in stages:
        span = 2 ** (s + 1)
        half = span // 2
        y = y.reshape(-1, DIM // span, span)
        left, right = y[..., :half], y[..., half:]
        th = a[s].reshape(1, DIM // span, half)
        c, sn = np.cos(th), np.sin(th)
        y = np.concatenate([c * left + sn * right, -sn * left + c * right], -1)
        y = y.reshape(-1, DIM)
    return y


def compose_transform(angles: np.ndarray) -> np.ndarray:
    """Full dense R (float32) with y = x @ R (kept for reference/tests)."""
    return np.ascontiguousarray(_compose(angles, range(NUM_STAGES)),
                                dtype=np.float32)


def build_bass(reps: int = 1):
    """reps>1 repeats the whole pipeline in one NEFF (for marginal timing)."""
    nc = bacc.Bacc(None, target_bir_lowering=False)
    x = nc.dram_tensor("x", [TOK_PER_CORE, DIM], F32, kind="ExternalInput")
    # rows 0-511: A = R_{1..9}[:512, :512]; rows 512-1023: B (bottom-right)
    wlow = nc.dram_tensor("wlow", [DIM, HALF], BF16, kind="ExternalInput")
    # stage-10 [cos | sin | cos | sin], replicated to 128 partitions on host
    cs2 = nc.dram_tensor("cs2", [128, 4 * HALF], F32, kind="ExternalInput")
    ident = nc.dram_tensor("ident", [128, 128], BF16, kind="ExternalInput")
    y = nc.dram_tensor("y", [TOK_PER_CORE, DIM], F32, kind="ExternalOutput")

    n_sub = TOK_PER_CORE // SUB  # 16 subtiles of 128 tokens

    in_chunks = [2, 2, 4, 4, 4]
    out_chunks = [4, 4, 4, 2, 2]
    assert sum(in_chunks) == n_sub and sum(out_chunks) == n_sub
    in_start = [sum(in_chunks[:i]) for i in range(len(in_chunks))]
    out_start = [sum(out_chunks[:i]) for i in range(len(out_chunks))]
    sub_to_in_chunk = {}
    for ci, (st, ln) in enumerate(zip(in_start, in_chunks)):
        for s in range(st, st + ln):
            sub_to_in_chunk[s] = ci
    sub_to_out_chunk = {}
    for ci, (st, ln) in enumerate(zip(out_start, out_chunks)):
        for s in range(st, st + ln):
            sub_to_out_chunk[s] = ci

    with tile.TileContext(nc) as tc:
        with (
            tc.tile_pool(name="const", bufs=1) as const_pool,
            tc.tile_pool(name="xin", bufs=3) as xin_pool,
            tc.tile_pool(name="xt", bufs=5) as xt_pool,
            tc.tile_pool(name="tmp", bufs=4) as tmp_pool,
            tc.tile_pool(name="yout", bufs=3) as yout_pool,
            tc.tile_pool(name="pst", bufs=3, space="PSUM") as pst_pool,
            tc.tile_pool(name="psz", bufs=2, space="PSUM") as psz_pool,
        ):
            ident_sb = const_pool.tile([128, 128], BF16, name="ident_sb")
            nc.sync.dma_start(ident_sb[:], ident[:])

            x_tiles = [None] * len(in_chunks)
            y_tiles = [None] * len(out_chunks)

            def load_chunk(ci):
                st, ln = in_start[ci], in_chunks[ci]
                x_tile = xin_pool.tile([128, ln * DIM], BF16, name="x_chunk",
                                       tag="x_chunk",
                                       padded_shape=[128, 4 * DIM])
                r0 = st * SUB
                # SWDGE: fp32 DRAM -> bf16 SBUF cast in the SDMA datapath.
                nc.gpsimd.dma_start(
                    x_tile[:, : ln * DIM].rearrange("p (s c) -> p s c", c=DIM),
                    x[r0 : r0 + ln * SUB, :].rearrange("(s p) c -> p s c", p=128),
                )
                x_tiles[ci] = x_tile

            load_chunk(0)
            first_load_done = True

            # A/B blocks: [128, kb*512] with a_sb[p, kb*512+j] = A[kb*128+p, j]
            a_sb = const_pool.tile([128, 4 * HALF], BF16, name="a_sb")
            b_sb = const_pool.tile([128, 4 * HALF], BF16, name="b_sb")
            for dst, r0 in ((a_sb, 0), (b_sb, HALF)):
                nc.sync.dma_start(
                    dst[:].rearrange("p (kb c) -> p kb c", c=HALF),
                    wlow[r0 : r0 + HALF, :].rearrange("(kb p) c -> p kb c", p=128),
                )
            cs_sb = const_pool.tile([128, 4 * HALF], F32, name="cs_sb")
            nc.sync.dma_start(cs_sb[:], cs2[:])

            xts = [None] * n_sub

            def emit_transpose(s):
                ci = sub_to_in_chunk[s]
                xcol = (s - in_start[ci]) * DIM
                x_tile = x_tiles[ci]
                # one bank: [128, 1024] bf16 = 2KB/partition
                ps_t = pst_pool.tile([128, DIM], BF16, name="ps_t", tag="ps_t")
                for kb in range(KB):
                    nc.tensor.transpose(
                        ps_t[:, kb * 128 : (kb + 1) * 128],
                        x_tile[:, xcol + kb * 128 : xcol + (kb + 1) * 128],
                        ident_sb,
                    )
                xt = xt_pool.tile([128, DIM], BF16, name="xt", tag="xt")
                nc.scalar.copy(xt[:], ps_t[:])
                xts[s] = xt

            def emit_subtile(s):
                co = sub_to_out_chunk[s]
                st, ln = out_start[co], out_chunks[co]
                if s == st:
                    y_tiles[co] = yout_pool.tile(
                        [128, ln * DIM], F32, name="y_chunk", tag="y_chunk",
                        padded_shape=[128, 4 * DIM],
                    )
                y_tile = y_tiles[co]
                ycol = (s - st) * DIM
                xt = xts[s]
                # zP[:, :512] = sum XT_kb^T @ A_kb ; zP[:, 512:] = .. @ B_kb
                zp = psz_pool.tile([128, DIM], F32, name="zp", tag="ps_z")
                for zi, w_sb in enumerate((a_sb, b_sb)):
                    for k in range(4):
                        kb = zi * 4 + k
                        nc.tensor.matmul(
                            zp[:, zi * 512 : (zi + 1) * 512],
                            xt[:, kb * 128 : (kb + 1) * 128],
                            w_sb[:, k * 512 : (k + 1) * 512],
                            start=(k == 0),
                            stop=(k == 3),
                        )
                # stage 10 products, one fused DVE op via broadcast AP:
                # t_all = [c*z0 | s*z0 | c*z1 | s*z1]
                t_all = tmp_pool.tile([128, 4 * 512], F32, name="t_all",
                                      tag="tmp")
                z_b = (zp[:].rearrange("p (h c) -> p h c", h=2)
                       .unsqueeze(2).to_broadcast((128, 2, 2, 512)))
                cs_b = cs_sb[:].rearrange("p (h r c) -> p h r c", h=2, r=2)
                t_b = t_all[:].rearrange("p (h r c) -> p h r c", h=2, r=2)
                nc.vector.tensor_tensor(t_b, z_b, cs_b, MULT)
                # y0 = c*z0 + s*z1 ; y1 = c*z1 - s*z0  (GpSimd)
                nc.gpsimd.tensor_tensor(
                    y_tile[:, ycol : ycol + 512],
                    t_all[:, 0:512], t_all[:, 1536:2048], ADD)
                nc.gpsimd.tensor_tensor(
                    y_tile[:, ycol + 512 : ycol + 1024],
                    t_all[:, 1024:1536], t_all[:, 512:1024], SUBTRACT)
                if s == st + ln - 1:
                    r0 = st * SUB
                    # y stores on the ACT HWDGE ring (x loads are SWDGE).
                    nc.scalar.dma_start(
                        y[r0 : r0 + ln * SUB, :].rearrange("(s p) c -> p s c", p=128),
                        y_tile[:, : ln * DIM].rearrange("p (s c) -> p s c", c=DIM),
                    )

            # Transposes run SKEW subtiles ahead of the matmuls so the PE
            # never waits on the ScalarE PSUM->SBUF evacuation of its own
            # transpose outputs.
            SKEW = 2
            for _rep in range(reps):
                if not first_load_done:
                    load_chunk(0)
                first_load_done = False
                for p in range(min(SKEW, n_sub)):
                    ci = sub_to_in_chunk[p]
                    if p == in_start[ci] and p > 0:
                        load_chunk(ci)
                    emit_transpose(p)
                for s in range(n_sub):
                    nxt = s + SKEW
                    if nxt < n_sub:
                        ci = sub_to_in_chunk[nxt]
                        if nxt == in_start[ci]:
                            load_chunk(ci)
                        emit_transpose(nxt)
                    emit_subtile(s)
    nc.compile()
    return nc


_NC_CACHE = None


def _get_nc():
    global _NC_CACHE
    if _NC_CACHE is None:
        _NC_CACHE = build_bass()
    return _NC_CACHE


def host_inputs(x: np.ndarray, angles: np.ndarray):
    """Per-core input maps (x fp32 shards; weights/coeffs precomputed)."""
    x = np.ascontiguousarray(np.asarray(x, dtype=np.float32))
    angles = np.asarray(angles)
    r9 = _compose(angles, range(NUM_STAGES - 1))
    wlow = np.concatenate([r9[:HALF, :HALF], r9[HALF:, HALF:]], axis=0)
    wlow = np.ascontiguousarray(wlow.astype(NP_BF16))
    c = np.cos(np.asarray(angles[NUM_STAGES - 1], dtype=np.float64))
    s = np.sin(np.asarray(angles[NUM_STAGES - 1], dtype=np.float64))
    cs = np.concatenate([c, s, c, s]).astype(np.float32)  # [c|s|c|s]
    cs2 = np.ascontiguousarray(np.broadcast_to(cs, (128, 4 * HALF)))
    ident = np.eye(128, dtype=np.float32).astype(NP_BF16)
    return [
        {
            "x": x[c_ * TOK_PER_CORE : (c_ + 1) * TOK_PER_CORE],
            "wlow": wlow,
            "cs2": cs2,
            "ident": ident,
        }
        for c_ in range(N_CORES)
    ]


def run(x: np.ndarray, angles: np.ndarray, trace: bool = False):
    """Run on 8 cores; returns (y_full, BassKernelResults)."""
    nc = _get_nc()
    in_maps = host_inputs(x, angles)
    res = run_bass_kernel_spmd(
        nc, in_maps, core_ids=list(range(N_CORES)), trace=trace
    )
    y = np.concatenate([res.results[c]["y"] for c in range(N_CORES)], axis=0)
    return y, res


def kernel(x: np.ndarray, angles: np.ndarray) -> np.ndarray:
    y, _ = run(x, angles, trace=False)
    return y


# revision 11
# speedup vs baseline: 1.6999x; 1.1837x over previous
"""Butterfly permuter kernel for Trainium2 (8 NeuronCores, SPMD data-parallel).

The reference applies 10 butterfly rotation stages along the feature axis
(dim=1024) of x [16384, 1024].  Stages 1-9 (spans 2..512) only mix within the
two 512-wide halves of the feature axis, so their composition is
blockdiag(A, B) with dense 512x512 blocks; stage 10 (span 1024) rotates
element j with element j+512 using per-column angles.  We therefore compute

    z0 = x[:, :512] @ A          (PE, bf16, fp32 PSUM accumulation)
    z1 = x[:, 512:] @ B
    y[:, :512] = c*z0 + s*z1     (DVE/GpSimd elementwise, c/s = cos/sin of
    y[:, 512:] = c*z1 - s*z0      angles[9], broadcast along tokens)

which halves the PE matmul work vs the dense 1024x1024 formulation (the PE
moving-operand stream at 1 elem/cycle was the bottleneck) and puts the last
stage on the otherwise-idle vector engines.  bf16 operands (rel-err budget
2e-2; this lands ~2e-3) with fp32 accumulation and fp32 stage-10 arithmetic.

Per core: x_shard [2048, 1024]
  - SWDGE-DMA x chunks with inline fp32->bf16 cast
  - PE-transpose each [128 tok, 128 dim] bf16 block (bf16 identity, bf16
    PSUM) to get X^T blocks, evacuate PSUM->SBUF on ScalarE
  - per 128-token subtile: 2 accumulation groups of 4 bf16 matmuls
    (z0 += XT_kb^T @ A_kb, z1 += XT_kb^T @ B_kb), N=512
  - stage 10: 4 DVE mults (PSUM x coeff -> SBUF temps) + 2 GpSimd add/sub
    (temps -> y tile, fp32)
  - HWDGE-DMA y out in chunks on the ACT ring

Inputs arrive full-size; sharding is across the token axis (2048 rows/core).
"""

import numpy as np

import concourse.bass as bass
import concourse.mybir as mybir
import concourse.tile as tile
from concourse import bacc
from concourse.bass_utils import run_bass_kernel_spmd

N_CORES = 8
DIM = 1024
NUM_STAGES = 10
N_TOKENS = 16384
TOK_PER_CORE = N_TOKENS // N_CORES  # 2048
SUB = 128  # tokens per subtile (partition dim)
KB = DIM // 128  # 8 contraction blocks
HALF = DIM // 2

F32 = mybir.dt.float32
BF16 = mybir.dt.bfloat16
NP_BF16 = mybir.dt.np(BF16)
MULT = mybir.AluOpType.mult
ADD = mybir.AluOpType.add
SUBTRACT = mybir.AluOpType.subtract


def _compose(angles: np.ndarray, stages) -> np.ndarray:
    y = np.eye(DIM, dtype=np.float64)
    a = np.asarray(angles, dtype=np.float64)
    for s in stages:
        span = 2 ** (s + 1)
        half = span // 2
        y = y.reshape(-1, DIM // span, span)
        left, right = y[..., :half], y[..., half:]
        th = a[s].reshape(1, DIM // span, half)
        c, sn = np.cos(th), np.sin(th)
        y = np.concatenate([c * left + sn * right, -sn * left + c * right], -1)
        y = y.reshape(-1, DIM)
    return y


def compose_transform(angles: np.ndarray) -> np.ndarray:
    """Full dense R (float32) with y = x @ R (kept for reference/tests)."""
    return np.ascontiguousarray(_compose(angles, range(NUM_STAGES)),
                                dtype=np.float32)


def build_bass(reps: int = 1):
    """reps>1 repeats the whole pipeline in one NEFF (for marginal timing)."""
    nc = bacc.Bacc(None, target_bir_lowering=False)
    x = nc.dram_tensor("x", [TOK_PER_CORE, DIM], F32, kind="ExternalInput")
    # rows 0-511: A = R_{1..9}[:512, :512]; rows 512-1023: B (bottom-right)
    wlow = nc.dram_tensor("wlow", [DIM, HALF], BF16, kind="ExternalInput")
    # stage-10 [cos | sin | cos | sin], replicated to 128 partitions on host
    cs2 = nc.dram_tensor("cs2", [128, 4 * HALF], F32, kind="ExternalInput")
    ident = nc.dram_tensor("ident", [128, 128], BF16, kind="ExternalInput")
    y = nc.dram_tensor("y", [TOK_PER_CORE, DIM], F32, kind="ExternalOutput")

    n_sub = TOK_PER_CORE // SUB  # 16 subtiles of 128 tokens
    total_sub = reps * n_sub

    in_chunks = [2, 2, 4, 4, 4]
    out_chunks = [4, 4, 4, 2, 2]
    assert sum(in_chunks) == n_sub and sum(out_chunks) == n_sub
    in_start = [sum(in_chunks[:i]) for i in range(len(in_chunks))]
    out_start = [sum(out_chunks[:i]) for i in range(len(out_chunks))]
    sub_to_in_chunk = {}
    for ci, (st, ln) in enumerate(zip(in_start, in_chunks)):
        for s in range(st, st + ln):
            sub_to_in_chunk[s] = ci
    sub_to_out_chunk = {}
    for ci, (st, ln) in enumerate(zip(out_start, out_chunks)):
        for s in range(st, st + ln):
            sub_to_out_chunk[s] = ci

    # x-chunk loads are emitted LOOK subtiles before their first consumer so
    # the SWDGE descriptor generation (Pool engine, strict FIFO) is never
    # stuck behind a full rep's worth of stage-10 adds at rep boundaries.
    LOOK = 6
    load_sched = []  # (emit_at_global_subtile, rep, ci), in emission order
    for rep in range(reps):
        for ci, st in enumerate(in_start):
            load_sched.append((max(0, rep * n_sub + st - LOOK), rep, ci))
    load_sched.sort(key=lambda t: t[0])

    with tile.TileContext(nc) as tc:
        with (
            tc.tile_pool(name="const", bufs=1) as const_pool,
            tc.tile_pool(name="xin", bufs=4) as xin_pool,
            tc.tile_pool(name="xt", bufs=5) as xt_pool,
            tc.tile_pool(name="tmp", bufs=4) as tmp_pool,
            tc.tile_pool(name="yout", bufs=3) as yout_pool,
            tc.tile_pool(name="pst", bufs=3, space="PSUM") as pst_pool,
            tc.tile_pool(name="psz", bufs=2, space="PSUM") as psz_pool,
        ):
            ident_sb = const_pool.tile([128, 128], BF16, name="ident_sb")
            nc.sync.dma_start(ident_sb[:], ident[:])

            x_tiles = {}  # (rep, ci) -> tile
            y_tiles = {}  # (rep, co) -> tile

            def load_chunk(rep, ci):
                st, ln = in_start[ci], in_chunks[ci]
                x_tile = xin_pool.tile([128, ln * DIM], BF16, name="x_chunk",
                                       tag="x_chunk",
                                       padded_shape=[128, 4 * DIM])
                r0 = st * SUB
                # SWDGE: fp32 DRAM -> bf16 SBUF cast in the SDMA datapath.
                nc.gpsimd.dma_start(
                    x_tile[:, : ln * DIM].rearrange("p (s c) -> p s c", c=DIM),
                    x[r0 : r0 + ln * SUB, :].rearrange("(s p) c -> p s c", p=128),
                )
                x_tiles[(rep, ci)] = x_tile

            # A/B blocks: [128, kb*512] with a_sb[p, kb*512+j] = A[kb*128+p, j]
            a_sb = const_pool.tile([128, 4 * HALF], BF16, name="a_sb")
            b_sb = const_pool.tile([128, 4 * HALF], BF16, name="b_sb")
            for dst, r0 in ((a_sb, 0), (b_sb, HALF)):
                nc.sync.dma_start(
                    dst[:].rearrange("p (kb c) -> p kb c", c=HALF),
                    wlow[r0 : r0 + HALF, :].rearrange("(kb p) c -> p kb c", p=128),
                )
            cs_sb = const_pool.tile([128, 4 * HALF], F32, name="cs_sb")
            nc.sync.dma_start(cs_sb[:], cs2[:])

            xts = [None] * n_sub

            def emit_transpose(s):
                ci = sub_to_in_chunk[s]
                xcol = (s - in_start[ci]) * DIM
                x_tile = x_tiles[ci]
                # one bank: [128, 1024] bf16 = 2KB/partition
                ps_t = pst_pool.tile([128, DIM], BF16, name="ps_t", tag="ps_t")
                for kb in range(KB):
                    nc.tensor.transpose(
                        ps_t[:, kb * 128 : (kb + 1) * 128],
                        x_tile[:, xcol + kb * 128 : xcol + (kb + 1) * 128],
                        ident_sb,
                    )
                xt = xt_pool.tile([128, DIM], BF16, name="xt", tag="xt")
                nc.scalar.copy(xt[:], ps_t[:])
                xts[s] = xt

            def emit_subtile(s):
                co = sub_to_out_chunk[s]
                st, ln = out_start[co], out_chunks[co]
                if s == st:
                    y_tiles[co] = yout_pool.tile(
                        [128, ln * DIM], F32, name="y_chunk", tag="y_chunk",
                        padded_shape=[128, 4 * DIM],
                    )
                y_tile = y_tiles[co]
                ycol = (s - st) * DIM
                xt = xts[s]
                # zP[:, :512] = sum XT_kb^T @ A_kb ; zP[:, 512:] = .. @ B_kb
                zp = psz_pool.tile([128, DIM], F32, name="zp", tag="ps_z")
                for zi, w_sb in enumerate((a_sb, b_sb)):
                    for k in range(4):
                        kb = zi * 4 + k
                        nc.tensor.matmul(
                            zp[:, zi * 512 : (zi + 1) * 512],
                            xt[:, kb * 128 : (kb + 1) * 128],
                            w_sb[:, k * 512 : (k + 1) * 512],
                            start=(k == 0),
                            stop=(k == 3),
                        )
                # stage 10 products, one fused DVE op via broadcast AP:
                # t_all = [c*z0 | s*z0 | c*z1 | s*z1]
                t_all = tmp_pool.tile([128, 4 * 512], F32, name="t_all",
                                      tag="tmp")
                z_b = (zp[:].rearrange("p (h c) -> p h c", h=2)
                       .unsqueeze(2).to_broadcast((128, 2, 2, 512)))
                cs_b = cs_sb[:].rearrange("p (h r c) -> p h r c", h=2, r=2)
                t_b = t_all[:].rearrange("p (h r c) -> p h r c", h=2, r=2)
                nc.vector.tensor_tensor(t_b, z_b, cs_b, MULT)
                # y0 = c*z0 + s*z1 ; y1 = c*z1 - s*z0  (GpSimd)
                nc.gpsimd.tensor_tensor(
                    y_tile[:, ycol : ycol + 512],
                    t_all[:, 0:512], t_all[:, 1536:2048], ADD)
                nc.gpsimd.tensor_tensor(
                    y_tile[:, ycol + 512 : ycol + 1024],
                    t_all[:, 1024:1536], t_all[:, 512:1024], SUBTRACT)
                if s == st + ln - 1:
                    r0 = st * SUB
                    # y stores on the ACT HWDGE ring (x loads are SWDGE).
                    nc.scalar.dma_start(
                        y[r0 : r0 + ln * SUB, :].rearrange("(s p) c -> p s c", p=128),
                        y_tile[:, : ln * DIM].rearrange("p (s c) -> p s c", c=DIM),
                    )

            # Transposes run SKEW subtiles ahead of the matmuls so the PE
            # never waits on the ScalarE PSUM->SBUF evacuation of its own
            # transpose outputs.
            SKEW = 2
            for _rep in range(reps):
                if not first_load_done:
                    load_chunk(0)
                first_load_done = False
                for p in range(min(SKEW, n_sub)):
                    ci = sub_to_in_chunk[p]
                    if p == in_start[ci] and p > 0:
                        load_chunk(ci)
                    emit_transpose(p)
                for s in range(n_sub):
                    nxt = s + SKEW
                    if nxt < n_sub:
                        ci = sub_to_in_chunk[nxt]
                        if nxt == in_start[ci]:
                            load_chunk(ci)
                        emit_transpose(nxt)
                    emit_subtile(s)
    nc.compile()
    return nc


_NC_CACHE = None


def _get_nc():
    global _NC_CACHE
    if _NC_CACHE is None:
        _NC_CACHE = build_bass()
    return _NC_CACHE


def host_inputs(x: np.ndarray, angles: np.ndarray):
    """Per-core input maps (x fp32 shards; weights/coeffs precomputed)."""
    x = np.ascontiguousarray(np.asarray(x, dtype=np.float32))
    angles = np.asarray(angles)
    r9 = _compose(angles, range(NUM_STAGES - 1))
    wlow = np.concatenate([r9[:HALF, :HALF], r9[HALF:, HALF:]], axis=0)
    wlow = np.ascontiguousarray(wlow.astype(NP_BF16))
    c = np.cos(np.asarray(angles[NUM_STAGES - 1], dtype=np.float64))
    s = np.sin(np.asarray(angles[NUM_STAGES - 1], dtype=np.float64))
    cs = np.concatenate([c, s, c, s]).astype(np.float32)  # [c|s|c|s]
    cs2 = np.ascontiguousarray(np.broadcast_to(cs, (128, 4 * HALF)))
    ident = np.eye(128, dtype=np.float32).astype(NP_BF16)
    return [
        {
            "x": x[c_ * TOK_PER_CORE : (c_ + 1) * TOK_PER_CORE],
            "wlow": wlow,
            "cs2": cs2,
            "ident": ident,
        }
        for c_ in range(N_CORES)
    ]


def run(x: np.ndarray, angles: np.ndarray, trace: bool = False):
    """Run on 8 cores; returns (y_full, BassKernelResults)."""
    nc = _get_nc()
    in_maps = host_inputs(x, angles)
    res = run_bass_kernel_spmd(
        nc, in_maps, core_ids=list(range(N_CORES)), trace=trace
    )
    y = np.concatenate([res.results[c]["y"] for c in range(N_CORES)], axis=0)
    return y, res


def kernel(x: np.ndarray, angles: np.ndarray) -> np.ndarray:
    y, _ = run(x, angles, trace=False)
    return y
